# revision 54
# baseline (speedup 1.0000x reference)
"""DecayAttention Trainium2 kernel (8 NeuronCores, SPMD).

Reference math (per batch b, head h):
    qkv = x @ qkv_w.T + qkv_b ; split to q,k,v [B,H,T,DH]
    s   = (q @ k.T) * DH**-0.5
    d   = exp(-softplus(lambda_raw[h]) * |i-j|)
    p   = softmax(s * d, axis=-1)
    out = (p @ v) reassembled, y = out @ proj_w.T + proj_b

Sharding: core c in 0..7 handles batch (c // 4) and heads 4*(c % 4) .. +4.
Each core computes a partial y^T (its 256 attention channels through the
projection); the host sums the 4 partials per batch and adds proj_b.

Key device-side structure (per core):
  - qkv projection computed transposed: qkvT[feat, t] so q,k live with the
    head dim on partitions (scores lhsT/rhs need [DH, t] layout).
  - Decay band: s*d is exactly representable as exp(-MHAT) for |i-j| beyond
    a lambda-dependent width; those columns contribute exp(-MHAT) * count to
    the softmax denominator and exp(-MHAT) * sum(v_far) to the numerator.
    Only a band of (2*KSIDE+1) 128-col tiles per 128-row block is computed.
  - Softmax uses a constant shift MHAT (softmax is shift invariant; inputs
    are bounded well below MHAT) so no row-max pass is needed.
  - AV runs transposed: out[65, queries] = [v | ones]^T @ strip, streaming
    wide query spans per key block, accumulated per 512-query group in one
    PSUM bank.  An indicator matmul initialises each group with the
    far-field numerator rows + far count (denominator), so no separate
    far-field pass and no PSUM pre-zero.  Row 64 is the softmax
    denominator; normalisation is reciprocal + partition_broadcast +
    one multiply straight into the projection-ready OT layout.
"""

import math
import numpy as np

from concourse import bacc, tile, mybir
from concourse.alu_op_type import AluOpType
from concourse.bass_utils import run_bass_kernel_spmd

F32 = mybir.dt.float32
BF16 = mybir.dt.bfloat16
AF = mybir.ActivationFunctionType
AX = mybir.AxisListType

NCORES = 8


def _ceil_div(a, b):
    return (a + b - 1) // b


# ---------------------------------------------------------------------------
# device program
# ---------------------------------------------------------------------------

def build_program(cfg):
    T = cfg["T"]          # tokens per batch (= tokens per core)
    C = cfg["C"]          # model dim
    DH = cfg["DH"]        # head dim (must be 64)
    Hpc = cfg["Hpc"]      # heads per core (even)
    KSIDE = cfg["KSIDE"]  # band half-width in 128-col tiles
    MHAT = cfg["MHAT"]    # constant softmax shift
    mm_fast = cfg.get("mm_fast", False)    # bf16 for big matmuls
    bf16_p = cfg.get("bf16_p", False)      # bf16 probs for AV
    zero_bias = cfg.get("zero_bias", False)
    share_decay = cfg.get("share_decay", False)

    assert DH == 64 and Hpc % 2 == 0 and T % 512 == 0 and C % 128 == 0
    P2 = Hpc // 2                # head-pair tiles for q/k/v
    NB = T // 128                # row/col blocks
    KC = C // 128                # qkv contraction chunks
    CPC = Hpc * DH               # per-core attention channels
    KCP = CPC // 128             # proj contraction chunks
    NO = C // 128                # proj output row blocks
    TCH = 512
    NTCH = T // TCH
    NF = 3 * P2                  # qkvT feature blocks
    BW = min((2 * KSIDE + 1), NB) * 128   # max band width (cols)
    GQ = 512                     # AV query-group width
    NG = T // GQ
    NBG = GQ // 128              # blocks per group
    EXPM = math.exp(-MHAT)
    assert KSIDE == 1 and NB <= 16

    mm_dt = BF16 if mm_fast else F32
    p_dt = BF16 if bf16_p else F32

    nc = bacc.Bacc("TRN2", target_bir_lowering=False, debug=False,
                   num_devices=NCORES)

    # ---- DRAM I/O ----
    # xT / wqkvT / projwT come host-prearranged into the exact SBUF layout
    # ([partition, ...] contiguous) so each DMA is 128 large contiguous
    # runs instead of thousands of 1KB packets (sync-engine issue time).
    xT_d = nc.dram_tensor("xT", [128, NTCH * KC * TCH], mm_dt,
                          kind="ExternalInput").ap()
    wqkvT_d = nc.dram_tensor("wqkvT", [128, NF * KC * 128], mm_dt,
                             kind="ExternalInput").ap()
    qkvb_d = nc.dram_tensor("qkvb2d", [128, NF], F32, kind="ExternalInput").ap()
    projwT_d = nc.dram_tensor("projwT", [128, KCP * C], mm_dt,
                              kind="ExternalInput").ap()
    neglam_d = nc.dram_tensor("neglam", [128, Hpc], F32, kind="ExternalInput").ap()
    jband_d = nc.dram_tensor("jband", [128, BW], F32, kind="ExternalInput").ap()
    rowcol_d = nc.dram_tensor("rowcol", [128, 1], F32, kind="ExternalInput").ap()
    ident_d = nc.dram_tensor("ident", [128, 128], F32, kind="ExternalInput").ap()
    ind_d = nc.dram_tensor("ind", [NB, T], p_dt, kind="ExternalInput").ap()
    nfexp_d = nc.dram_tensor("nfexp", [NB, 1], p_dt, kind="ExternalInput").ap()
    yT_d = nc.dram_tensor("yT", [C, T], p_dt, kind="ExternalOutput").ap()

    def band_of(ki):
        return max(0, ki - KSIDE), min(NB - 1, ki + KSIDE)

    with tile.TileContext(nc) as tc:
        with (
            tc.tile_pool(name="persist", bufs=1) as persist,
            tc.tile_pool(name="consts", bufs=1) as consts,
            tc.tile_pool(name="work", bufs=3) as work,
            tc.tile_pool(name="stats", bufs=8) as stats,
            tc.tile_pool(name="sw", bufs=4) as sw,
            tc.tile_pool(name="stage", bufs=3) as stage_pool,
            tc.tile_pool(name="xt", bufs=2) as xt_pool,
            tc.tile_pool(name="strips", bufs=2 * NB + 2) as strips_pool,
            tc.tile_pool(name="psumS", bufs=4, space="PSUM") as psumS,
            tc.tile_pool(name="psumAV", bufs=2, space="PSUM") as psumAV,
            tc.tile_pool(name="psumSm", bufs=2, space="PSUM") as psumSm,
        ):
            # ---------------- constants ----------------
            # critical path: qkv weight chunks + first x chunk on the sync
            # queue; everything else issued from the idle gpsimd queue so
            # the first matmul is not stuck behind a pile of small DMAs.
            wbig = consts.tile([128, NF, KC, 128], mm_dt, tag="wbig")
            WCH = KC * 128
            nc.sync.dma_start(wbig[:, 0, :, :], wqkvT_d[:, 0:WCH])
            w_sb = {(f, kc): wbig[:, f, kc, :]
                    for f in range(NF) for kc in range(KC)}

            ident = consts.tile([128, 128], F32, tag="ident")
            nc.gpsimd.dma_start(ident[:], ident_d[:])
            if mm_fast or bf16_p:
                ident_b = consts.tile([128, 128], BF16, tag="ident_b")
                nc.vector.tensor_copy(ident_b[:], ident[:])
            else:
                ident_b = ident
            ident_m = ident_b if mm_fast else ident
            mhat_b = consts.tile([128, 1], F32, tag="mhat_b")
            nc.gpsimd.memset(mhat_b[:], float(-MHAT))
            if not zero_bias:
                qkvb = consts.tile([128, NF], F32, tag="qkvb")
                nc.gpsimd.dma_start(qkvb[:], qkvb_d[:])
            neglam = consts.tile([128, Hpc], F32, tag="neglam")
            nc.gpsimd.dma_start(neglam[:], neglam_d[:])
            jband = consts.tile([128, BW], F32, tag="jband")
            nc.gpsimd.dma_start(jband[:], jband_d[:])
            rowcol = consts.tile([128, 1], F32, tag="rowcol")
            nc.gpsimd.dma_start(rowcol[:], rowcol_d[:])
            indt = consts.tile([NB, T], p_dt, tag="indt")
            nc.gpsimd.dma_start(indt[:], ind_d[:])
            nfexp = consts.tile([NB, 1], p_dt, tag="nfexp")
            nc.gpsimd.dma_start(nfexp[:], nfexp_d[:])
            pwbig = consts.tile([128, KCP, C], mm_dt, tag="pwbig")
            nc.gpsimd.dma_start(pwbig[:], projwT_d[:])
            pw_sb = {(o, kc): pwbig[:, kc, o * 128:(o + 1) * 128]
                     for o in range(NO) for kc in range(KCP)}
            XCH = KC * TCH

            # persistent activations
            qkvT = [persist.tile([128, T], mm_dt, tag=f"qkvT{f}",
                                 name=f"qkvT{f}")
                    for f in range(NF)]
            OT = [persist.tile([128, T], mm_dt, tag=f"OT{k}", name=f"OT{k}")
                  for k in range(KCP)]

            # PSUM->SBUF copies alternate vector/scalar (gpsimd cannot
            # read PSUM); some scalar table thrash is accepted to keep the
            # qkv psum banks draining fast
            copy_flip = [0]

            def psum_copy(dst, src):
                if copy_flip[0] % 2 == 0:
                    nc.vector.tensor_copy(dst, src)
                else:
                    nc.scalar.copy(dst, src)
                copy_flip[0] += 1

            # ---------------- phase 1: qkvT = Wqkv @ x^T (+bias) ------------
            # tch-major with a rotating x chunk: compute all NF features for
            # one 512-token chunk while the next chunk's DMA is in flight.
            # Score strips for key blocks whose q/k inputs are complete are
            # interleaved after each chunk (range-based hazards make them
            # depend only on the written slices), so the vector/scalar
            # sd-mult + exp pipeline runs during phase 1 instead of after.
            def phase1_tch(tch):
                xt = xt_pool.tile([128, KC, TCH], mm_dt, tag="xt")
                nc.sync.dma_start(
                    xt[:], xT_d[:, tch * XCH:(tch + 1) * XCH])
                if tch == 0:
                    for f in range(1, NF):
                        nc.sync.dma_start(wbig[:, f, :, :],
                                          wqkvT_d[:, f * WCH:(f + 1) * WCH])
                for f in range(NF):
                    ps = psumS.tile([128, TCH], F32, tag="S")
                    for kc in range(KC):
                        nc.tensor.matmul(
                            ps[:],
                            w_sb[(f, kc)],
                            xt[:, kc, :],
                            start=(kc == 0), stop=(kc == KC - 1))
                    dst = qkvT[f][:, tch * TCH:(tch + 1) * TCH]
                    if zero_bias:
                        psum_copy(dst, ps[:])
                    else:
                        nc.scalar.activation(
                            dst, ps[:], AF.Identity, bias=qkvb[:, f:f + 1])

            # ---------------- phase 2: attention per head pair --------------
            shared_decay = [None]

            def make_decay(lh):
                """Decay band for head lh — consts only, can run early."""
                if share_decay and shared_decay[0] is not None:
                    return shared_decay[0]
                dist = work.tile([128, BW], F32, tag="dist",
                                 name=f"dist{lh}")
                nc.vector.tensor_scalar(
                    dist[:], jband[:], rowcol[:], None,
                    AluOpType.subtract)
                nc.scalar.activation(dist[:], dist[:], AF.Abs)
                decay = work.tile([128, BW], F32, tag="decay",
                                  name=f"decay{lh}")
                nc.scalar.activation(decay[:], dist[:], AF.Exp,
                                     scale=neglam[:, lh:lh + 1])
                if share_decay:
                    shared_decay[0] = decay
                return decay

            def head_ctx(lh, decay):
                """Per-head far-field init rows (sfT65): needs v from qkvT."""
                pr, par = lh // 2, lh % 2
                pb = par * 64
                vv = qkvT[2 * P2 + pr]

                # far-field row sums: sfar[ch, k] = EXPM * (tot - band sum)
                vcs = sw.tile([64, NB], F32, tag="vcs", name=f"vcs{lh}")
                nc.vector.tensor_reduce(
                    vcs[:], vv[pb:pb + 64, :].rearrange(
                        "p (k t) -> p k t", k=NB),
                    AX.X, AluOpType.add)
                pad = sw.tile([64, NB + 2 * KSIDE], F32, tag="pad",
                              name=f"pad{lh}")
                nc.gpsimd.memset(pad[:], 0.0)
                nc.gpsimd.tensor_copy(pad[:, KSIDE:KSIDE + NB], vcs[:])
                b5 = sw.tile([64, NB], F32, tag="b5", name=f"b5{lh}")
                nc.gpsimd.tensor_tensor(
                    b5[:], pad[:, 0:NB], pad[:, 1:1 + NB], AluOpType.add)
                for d in range(2, 2 * KSIDE + 1):
                    nc.gpsimd.tensor_tensor(
                        b5[:], b5[:], pad[:, d:d + NB], AluOpType.add)
                tot = stats.tile([64, 1], F32, tag="tot", name=f"tot{lh}")
                nc.vector.tensor_reduce(tot[:], vcs[:], AX.X, AluOpType.add)
                sfar = sw.tile([64, NB], F32, tag="sfar", name=f"sfar{lh}")
                nc.gpsimd.tensor_scalar(
                    sfar[:], b5[:], tot[:], -EXPM,
                    AluOpType.subtract, AluOpType.mult)
                sfT_ps = psumSm.tile([16, 64], F32, tag="small",
                                     name=f"sfT_ps{lh}")
                nc.tensor.transpose(
                    sfT_ps[:NB, :], sfar[:], ident[0:64, 0:64])
                sfT65 = sw.tile([NB, 65], p_dt, tag="sfT65",
                                name=f"sfT65{lh}")
                nc.vector.tensor_copy(sfT65[:, 0:64], sfT_ps[:NB, :])
                nc.gpsimd.tensor_copy(sfT65[:, 64:65], nfexp[:])
                return dict(lh=lh, pb=pb, sfT65=sfT65)

            def make_vnat(pr):
                """v^T for the head pair, 65-col groups [v64 | ones] per
                (block, head): [128, NB*130]."""
                vv = qkvT[2 * P2 + pr]
                vn = work.tile([128, NB * 130], p_dt, tag="vnat",
                               name=f"vnat{pr}")
                nc.gpsimd.memset(vn[:].rearrange(
                    "p (k e) -> p k e", e=65)[:, :, 64:65], 1.0)
                for k in range(NB):
                    tp = psumSm.tile([128, 128], mm_dt, tag="small",
                                     name=f"vtp{pr}_{k}")
                    nc.tensor.transpose(
                        tp[:], vv[:, k * 128:(k + 1) * 128], ident_m)
                    psum_copy(
                        vn[:, k * 130:k * 130 + 130].rearrange(
                            "p (b e) -> p b e", b=2)[:, :, 0:64],
                        tp[:].rearrange("p (b e) -> p b e", b=2))
                return vn

            def strip_pair(decs, pr, c):
                """exp((q.k^T)*decay - MHAT) for key block c, both heads.

                Layout [128 keys, 2*BW]: head h2 at columns h2*BW.."""
                til, tir = band_of(c)
                w = (tir - til + 1) * 128
                off = (til - (c - KSIDE)) * 128
                qq = qkvT[pr]
                kk = qkvT[P2 + pr]
                sdp = work.tile([128, 2 * BW], F32, tag="sdp",
                                name=f"sdp_{pr}_{c}")
                stp = strips_pool.tile([128, 2 * BW], p_dt, tag="strip",
                                       name=f"strip_{pr}_{c}")
                for h2 in range(2):
                    pb = h2 * 64
                    st_ps = psumS.tile([128, TCH], F32, tag="S",
                                       name=f"st_{pr}_{c}_{h2}")
                    nc.tensor.matmul(
                        st_ps[:, :w],
                        kk[pb:pb + 64, c * 128:(c + 1) * 128],
                        qq[pb:pb + 64, til * 128:til * 128 + w],
                        start=True, stop=True)
                    nc.vector.tensor_tensor(
                        sdp[:, h2 * BW:h2 * BW + w], st_ps[:, :w],
                        decs[h2][:, off:off + w], AluOpType.mult)
                if w == BW:
                    nc.scalar.activation(stp[:], sdp[:], AF.Exp,
                                         bias=mhat_b[:])
                else:
                    for h2 in range(2):
                        nc.scalar.activation(
                            stp[:, h2 * BW:h2 * BW + w],
                            sdp[:, h2 * BW:h2 * BW + w], AF.Exp,
                            bias=mhat_b[:])
                return dict(tile=stp, til=til)

            def av_group(hc, h2, vn, strips, g):
                """out^T[65, GQ] for query group g: init with far rows via
                indicator matmul, accumulate band blocks, normalise into
                OT[channels, T]."""
                lh = hc["lh"]
                avps = psumAV.tile([65, GQ], F32, tag="AV",
                                   name=f"av_{lh}_{g}")
                nc.tensor.matmul(
                    avps[:], hc["sfT65"][:],
                    indt[:, g * GQ:(g + 1) * GQ],
                    start=True, stop=False)
                i0 = g * NBG
                cs = [c for c in range(max(0, i0 - KSIDE),
                                       min(NB - 1, i0 + NBG - 1 + KSIDE) + 1)]
                for c in cs:
                    til, tir = band_of(c)
                    ilo, ihi = max(til, i0), min(tir, i0 + NBG - 1)
                    if ilo > ihi:
                        continue
                    sp = strips[c]
                    s0 = h2 * BW + (ilo - sp["til"]) * 128
                    s1 = h2 * BW + (ihi + 1 - sp["til"]) * 128
                    nc.tensor.matmul(
                        avps[:, (ilo - i0) * 128:(ihi + 1 - i0) * 128],
                        vn[:, c * 130 + h2 * 65:c * 130 + h2 * 65 + 65],
                        sp["tile"][:, s0:s1],
                        start=False, stop=(c == cs[-1]))
                # plain DVE reciprocal is ~6ns/element serial — far too slow
                # for a [1, GQ] row; the approx variant (~18 bits) is plenty
                # against the 2e-2 gate and z is a well-scaled positive.
                zrow = stats.tile([1, GQ], F32, tag="zrow",
                                  name=f"z_{lh}_{g}")
                nc.scalar.copy(zrow[:], avps[64:65, :])
                rzrow = stats.tile([1, GQ], F32, tag="rzrow",
                                   name=f"rz_{lh}_{g}")
                nc.vector.reciprocal_approx_fast(rzrow[:], zrow[:])
                rzb = stage_pool.tile([64, GQ], F32, tag="rzb",
                                      name=f"rzb_{lh}_{g}")
                nc.gpsimd.partition_broadcast(rzb[:], rzrow[:], channels=64)
                kc, hh = divmod(lh, 2)
                nc.vector.tensor_tensor(
                    OT[kc][hh * 64:(hh + 1) * 64, g * GQ:(g + 1) * GQ],
                    avps[0:64, :], rzb[:], AluOpType.mult)

            # ---------------- phase 3: yT = projW^T @ OT --------------------
            def proj_group(g):
                for o in range(NO):
                    ps = psumS.tile([128, GQ], F32, tag="S")
                    for kc in range(KCP):
                        nc.tensor.matmul(
                            ps[:],
                            pw_sb[(o, kc)],
                            OT[kc][:, g * GQ:(g + 1) * GQ],
                            start=(kc == 0), stop=(kc == KCP - 1))
                    st = stage_pool.tile([128, GQ], p_dt, tag="stage")
                    if o % 2 == 0:
                        nc.scalar.copy(st[:], ps[:])
                    else:
                        nc.vector.tensor_copy(st[:], ps[:])
                    eng = nc.sync if o % 2 == 0 else nc.gpsimd
                    eng.dma_start(
                        yT_d[o * 128:(o + 1) * 128, g * GQ:(g + 1) * GQ],
                        st[:])

            # ---------------- emission ----------------
            # qkv chunks and score strips interleave (strips for block c run
            # as soon as its q/k token chunks exist), so sd-mult + exp flow
            # through vector/scalar during phase 1.  AV groups and the
            # projection then interleave per query group so output DMAs
            # start early and the tail is short.
            decays = [make_decay(lh) for lh in range(Hpc)]
            all_strips = [dict() for _ in range(P2)]
            cdone = 0
            pair_data = []
            for tch in range(NTCH):
                phase1_tch(tch)
                if tch == NTCH - 1:
                    # vnat + far-field rows first: the gpsimd sfar chain and
                    # the vnat transposes run while vector/scalar drain the
                    # final strips, so AV groups are not gated on them
                    for pr in range(P2):
                        hcs = [head_ctx(2 * pr, decays[2 * pr]),
                               head_ctx(2 * pr + 1, decays[2 * pr + 1])]
                        vn = make_vnat(pr)
                        pair_data.append((hcs, vn, all_strips[pr]))
                cmax = NB if tch == NTCH - 1 else 4 * (tch + 1) - 1
                for c in range(cdone, cmax):
                    for pr in range(P2):
                        all_strips[pr][c] = strip_pair(
                            decays[2 * pr:2 * pr + 2], pr, c)
                cdone = cmax
            for g in range(NG):
                for hcs, vn, strips in pair_data:
                    for h2, hc in enumerate(hcs):
                        av_group(hc, h2, vn, strips, g)
                proj_group(g)

    nc.compile()
    return nc


# ---------------------------------------------------------------------------
# host side
# ---------------------------------------------------------------------------

def _softplus(x):
    x = np.asarray(x, np.float64)
    return np.log1p(np.exp(-np.abs(x))) + np.maximum(x, 0.0)


def make_host_data(x, qkv_w, qkv_b, proj_w, proj_b, lambda_raw,
                   ncores=NCORES, mm_fast=True, bf16_p=True):
    """Returns (cfg, in_maps, assemble(results) -> y)."""
    x = np.asarray(x, np.float32)
    qkv_w = np.asarray(qkv_w, np.float32)
    qkv_b = np.asarray(qkv_b, np.float32)
    proj_w = np.asarray(proj_w, np.float32)
    proj_b = np.asarray(proj_b, np.float32)
    lambda_raw = np.asarray(lambda_raw, np.float32)

    B, T, C = x.shape
    H = lambda_raw.shape[0]
    DH = C // H
    NCH = ncores // B
    Hpc = H // NCH
    P2 = Hpc // 2
    NB = T // 128
    scale = DH ** -0.5

    lam = _softplus(lambda_raw)

    # constant softmax shift: bound on |s| (sampled, with generous margin)
    rng = np.random.default_rng(0)
    idx = rng.choice(B * T, size=min(256, B * T), replace=False)
    xs = x.reshape(B * T, C)[idx]
    qs = (xs @ qkv_w[:C].T).reshape(-1, H, DH)
    ks = (xs @ qkv_w[C:2 * C].T).reshape(-1, H, DH)
    smax = 0.0
    for h in range(H):
        smax = max(smax, float(np.abs(
            (qs[:, h] * scale) @ ks[:, h].T).max()))
    MHAT = float(max(16.0, math.ceil(2.5 * smax + 8.0)))

    lam_min = float(lam.min())
    # band cutoff: beyond the band, |s*d| <= MHAT*exp(-lam*dist) <= 1e-4,
    # so exp(s*d - MHAT) deviates from the far-field exp(-MHAT) by <= 1e-4
    # relative — far below the 2e-2 gate.
    thresh = math.log(max(MHAT, 16.0) / 1e-4)
    KSIDE = _ceil_div(max(1, int(math.ceil(thresh / lam_min)) - 1), 128)
    KSIDE = max(1, KSIDE)
    assert KSIDE == 1, "band wider than 1 tile not supported by this kernel"
    EXPM = math.exp(-MHAT)

    zero_bias = not (qkv_b.any())
    share_decay = bool(np.all(lam == lam[0]))

    cfg = dict(T=T, C=C, DH=DH, Hpc=Hpc, KSIDE=KSIDE, MHAT=MHAT,
               mm_fast=mm_fast, bf16_p=bf16_p, zero_bias=zero_bias,
               share_decay=share_decay)

    NF = 3 * P2
    BW = min(2 * KSIDE + 1, NB) * 128
    pcol = np.arange(128, dtype=np.float32)
    jb = np.broadcast_to(
        np.arange(BW, dtype=np.float32) - KSIDE * 128, (128, BW))
    rc = pcol[:, None]

    # AV group-init indicator [NB, T] and per-block far counts
    ind = np.zeros((NB, T), np.float32)
    for k in range(NB):
        ind[k, k * 128:(k + 1) * 128] = 1.0
    nfar = np.empty((NB, 1), np.float32)
    for i in range(NB):
        kl, kr = max(0, i - KSIDE), min(NB - 1, i + KSIDE)
        nfar[i, 0] = (T - (kr - kl + 1) * 128) * EXPM

    if mm_fast or bf16_p:
        import ml_dtypes
    mm_np = ml_dtypes.bfloat16 if mm_fast else np.float32
    p_np = ml_dtypes.bfloat16 if bf16_p else np.float32

    KC = C // 128
    TCH = 512
    NTCH = T // TCH
    KCP = (Hpc * DH) // 128

    in_maps = []
    for c in range(ncores):
        b, g = divmod(c, NCH)
        hbase = g * Hpc
        # x^T prearranged to SBUF layout [128, (tch kc tcol)] so the device
        # DMA is contiguous per partition
        xT = (x[b].T.reshape(KC, 128, NTCH, TCH).transpose(1, 2, 0, 3)
              .reshape(128, -1).astype(mm_np))
        xT = np.ascontiguousarray(xT)
        wblocks, bblocks = [], []
        for f in range(NF):
            ftype, pr = divmod(f, P2)
            r0 = ftype * C + (hbase + 2 * pr) * DH
            wf = qkv_w[r0:r0 + 128]          # [128, C]
            bf = qkv_b[r0:r0 + 128]
            if ftype == 0:                    # fold score scale into q
                wf = wf * scale
                bf = bf * scale
            wblocks.append(wf.T)
            bblocks.append(bf)
        wqkvT = np.concatenate(wblocks, 1)   # [C, NF*128]
        # -> [128, (f kc m)]
        wqkvT = (wqkvT.reshape(KC, 128, NF, 128).transpose(1, 2, 0, 3)
                 .reshape(128, -1).astype(mm_np))
        wqkvT = np.ascontiguousarray(wqkvT)
        qkvb2d = np.stack(bblocks, 1).astype(np.float32)
        projwT = proj_w[:, hbase * DH:hbase * DH + Hpc * DH].T  # [CPC, C]
        projwT = (projwT.reshape(KCP, 128, C).transpose(1, 0, 2)
                  .reshape(128, -1).astype(mm_np))
        projwT = np.ascontiguousarray(projwT)
        nl = np.broadcast_to(
            (-lam[hbase:hbase + Hpc]).astype(np.float32), (128, Hpc))
        in_maps.append({
            "xT": xT,
            "wqkvT": wqkvT,
            "qkvb2d": np.ascontiguousarray(qkvb2d),
            "projwT": projwT,
            "neglam": np.ascontiguousarray(nl),
            "jband": np.ascontiguousarray(jb),
            "rowcol": np.ascontiguousarray(rc),
            "ident": np.eye(128, dtype=np.float32),
            "ind": ind.astype(p_np),
            "nfexp": nfar.astype(p_np),
        })

    def assemble(results):
        y = np.zeros((B, T, C), np.float32)
        for c in range(ncores):
            b = c // NCH
            y[b] += np.asarray(results[c]["yT"], np.float32).T
        y += proj_b[None, None, :]
        return y

    return cfg, in_maps, assemble


_PROGRAM_CACHE = {}


def kernel(x, qkv_w, qkv_b, proj_w, proj_b, lambda_raw,
           mm_fast=True, bf16_p=True, trace=False):
    cfg, in_maps, assemble = make_host_data(
        x, qkv_w, qkv_b, proj_w, proj_b, lambda_raw,
        mm_fast=mm_fast, bf16_p=bf16_p)
    key = tuple(sorted(cfg.items()))
    if key not in _PROGRAM_CACHE:
        _PROGRAM_CACHE[key] = build_program(cfg)
    nc = _PROGRAM_CACHE[key]
    res = run_bass_kernel_spmd(nc, in_maps, core_ids=list(range(NCORES)),
                               trace=trace)
    out = assemble(res.results)
    if trace:
        kernel.last_results = res
    return out


# revision 55
# speedup vs baseline: 1.0427x; 1.0427x over previous
"""DecayAttention Trainium2 kernel (8 NeuronCores, SPMD).

Reference math (per batch b, head h):
    qkv = x @ qkv_w.T + qkv_b ; split to q,k,v [B,H,T,DH]
    s   = (q @ k.T) * DH**-0.5
    d   = exp(-softplus(lambda_raw[h]) * |i-j|)
    p   = softmax(s * d, axis=-1)
    out = (p @ v) reassembled, y = out @ proj_w.T + proj_b

Sharding: core c in 0..7 handles batch (c // 4) and heads 4*(c % 4) .. +4.
Each core computes a partial y^T (its 256 attention channels through the
projection); the host sums the 4 partials per batch and adds proj_b.

Key device-side structure (per core):
  - qkv projection computed transposed: qkvT[feat, t] so q,k live with the
    head dim on partitions (scores lhsT/rhs need [DH, t] layout).
  - Decay band: s*d is exactly representable as exp(-MHAT) for |i-j| beyond
    a lambda-dependent width; those columns contribute exp(-MHAT) * count to
    the softmax denominator and exp(-MHAT) * sum(v_far) to the numerator.
    Only a band of (2*KSIDE+1) 128-col tiles per 128-row block is computed.
  - Softmax uses a constant shift MHAT (softmax is shift invariant; inputs
    are bounded well below MHAT) so no row-max pass is needed.
  - AV runs transposed: out[65, queries] = [v | ones]^T @ strip, streaming
    wide query spans per key block, accumulated per 512-query group in one
    PSUM bank.  An indicator matmul initialises each group with the
    far-field numerator rows + far count (denominator), so no separate
    far-field pass and no PSUM pre-zero.  Row 64 is the softmax
    denominator; normalisation is reciprocal + partition_broadcast +
    one multiply straight into the projection-ready OT layout.
"""

import math
import numpy as np

from concourse import bacc, tile, mybir
from concourse.alu_op_type import AluOpType
from concourse.bass_utils import run_bass_kernel_spmd

F32 = mybir.dt.float32
BF16 = mybir.dt.bfloat16
AF = mybir.ActivationFunctionType
AX = mybir.AxisListType

NCORES = 8


def _ceil_div(a, b):
    return (a + b - 1) // b


# ---------------------------------------------------------------------------
# device program
# ---------------------------------------------------------------------------

def build_program(cfg):
    T = cfg["T"]          # tokens per batch (= tokens per core)
    C = cfg["C"]          # model dim
    DH = cfg["DH"]        # head dim (must be 64)
    Hpc = cfg["Hpc"]      # heads per core (even)
    KSIDE = cfg["KSIDE"]  # band half-width in 128-col tiles
    MHAT = cfg["MHAT"]    # constant softmax shift
    mm_fast = cfg.get("mm_fast", False)    # bf16 for big matmuls
    bf16_p = cfg.get("bf16_p", False)      # bf16 probs for AV
    zero_bias = cfg.get("zero_bias", False)
    share_decay = cfg.get("share_decay", False)

    assert DH == 64 and Hpc % 2 == 0 and T % 512 == 0 and C % 128 == 0
    P2 = Hpc // 2                # head-pair tiles for q/k/v
    NB = T // 128                # row/col blocks
    KC = C // 128                # qkv contraction chunks
    CPC = Hpc * DH               # per-core attention channels
    KCP = CPC // 128             # proj contraction chunks
    NO = C // 128                # proj output row blocks
    TCH = 512
    NTCH = T // TCH
    NF = 3 * P2                  # qkvT feature blocks
    BW = min((2 * KSIDE + 1), NB) * 128   # max band width (cols)
    GQ = 512                     # AV query-group width
    NG = T // GQ
    NBG = GQ // 128              # blocks per group
    EXPM = math.exp(-MHAT)
    assert KSIDE == 1 and NB <= 16

    mm_dt = BF16 if mm_fast else F32
    p_dt = BF16 if bf16_p else F32

    nc = bacc.Bacc("TRN2", target_bir_lowering=False, debug=False,
                   num_devices=NCORES)

    # ---- DRAM I/O ----
    # xT / wqkvT / projwT come host-prearranged into the exact SBUF layout
    # ([partition, ...] contiguous) so each DMA is 128 large contiguous
    # runs instead of thousands of 1KB packets (sync-engine issue time).
    xT_d = nc.dram_tensor("xT", [128, NTCH * KC * TCH], mm_dt,
                          kind="ExternalInput").ap()
    wqkvT_d = nc.dram_tensor("wqkvT", [128, NF * KC * 128], mm_dt,
                             kind="ExternalInput").ap()
    qkvb_d = nc.dram_tensor("qkvb2d", [128, NF], F32, kind="ExternalInput").ap()
    projwT_d = nc.dram_tensor("projwT", [128, KCP * C], mm_dt,
                              kind="ExternalInput").ap()
    neglam_d = nc.dram_tensor("neglam", [128, Hpc], F32, kind="ExternalInput").ap()
    jband_d = nc.dram_tensor("jband", [128, BW], F32, kind="ExternalInput").ap()
    rowcol_d = nc.dram_tensor("rowcol", [128, 1], F32, kind="ExternalInput").ap()
    ident_d = nc.dram_tensor("ident", [128, 128], F32, kind="ExternalInput").ap()
    ind_d = nc.dram_tensor("ind", [NB, T], p_dt, kind="ExternalInput").ap()
    nfexp_d = nc.dram_tensor("nfexp", [NB, 1], p_dt, kind="ExternalInput").ap()
    yT_d = nc.dram_tensor("yT", [C, T], p_dt, kind="ExternalOutput").ap()

    def band_of(ki):
        return max(0, ki - KSIDE), min(NB - 1, ki + KSIDE)

    with tile.TileContext(nc) as tc:
        with (
            tc.tile_pool(name="persist", bufs=1) as persist,
            tc.tile_pool(name="consts", bufs=1) as consts,
            tc.tile_pool(name="work", bufs=3) as work,
            tc.tile_pool(name="stats", bufs=8) as stats,
            tc.tile_pool(name="sw", bufs=4) as sw,
            tc.tile_pool(name="stage", bufs=3) as stage_pool,
            tc.tile_pool(name="xt", bufs=2) as xt_pool,
            tc.tile_pool(name="strips", bufs=2 * NB + 2) as strips_pool,
            tc.tile_pool(name="psumS", bufs=3, space="PSUM") as psumS,
            tc.tile_pool(name="psumAV", bufs=3, space="PSUM") as psumAV,
            tc.tile_pool(name="psumSm", bufs=2, space="PSUM") as psumSm,
        ):
            # ---------------- constants ----------------
            # critical path: qkv weight chunks + first x chunk on the sync
            # queue; everything else issued from the idle gpsimd queue so
            # the first matmul is not stuck behind a pile of small DMAs.
            wbig = consts.tile([128, NF, KC, 128], mm_dt, tag="wbig")
            WCH = KC * 128
            nc.sync.dma_start(wbig[:, 0, :, :], wqkvT_d[:, 0:WCH])
            w_sb = {(f, kc): wbig[:, f, kc, :]
                    for f in range(NF) for kc in range(KC)}

            ident = consts.tile([128, 128], F32, tag="ident")
            nc.gpsimd.dma_start(ident[:], ident_d[:])
            if mm_fast or bf16_p:
                ident_b = consts.tile([128, 128], BF16, tag="ident_b")
                nc.vector.tensor_copy(ident_b[:], ident[:])
            else:
                ident_b = ident
            ident_m = ident_b if mm_fast else ident
            mhat_b = consts.tile([128, 1], F32, tag="mhat_b")
            nc.gpsimd.memset(mhat_b[:], float(-MHAT))
            if not zero_bias:
                qkvb = consts.tile([128, NF], F32, tag="qkvb")
                nc.gpsimd.dma_start(qkvb[:], qkvb_d[:])
            neglam = consts.tile([128, Hpc], F32, tag="neglam")
            nc.gpsimd.dma_start(neglam[:], neglam_d[:])
            jband = consts.tile([128, BW], F32, tag="jband")
            nc.gpsimd.dma_start(jband[:], jband_d[:])
            rowcol = consts.tile([128, 1], F32, tag="rowcol")
            nc.gpsimd.dma_start(rowcol[:], rowcol_d[:])
            indt = consts.tile([NB, T], p_dt, tag="indt")
            nc.gpsimd.dma_start(indt[:], ind_d[:])
            nfexp = consts.tile([NB, 1], p_dt, tag="nfexp")
            nc.gpsimd.dma_start(nfexp[:], nfexp_d[:])
            pwbig = consts.tile([128, KCP, C], mm_dt, tag="pwbig")
            nc.gpsimd.dma_start(pwbig[:], projwT_d[:])
            pw_sb = {(o, kc): pwbig[:, kc, o * 128:(o + 1) * 128]
                     for o in range(NO) for kc in range(KCP)}
            XCH = KC * TCH

            # persistent activations
            qkvT = [persist.tile([128, T], mm_dt, tag=f"qkvT{f}",
                                 name=f"qkvT{f}")
                    for f in range(NF)]
            OT = [persist.tile([128, T], mm_dt, tag=f"OT{k}", name=f"OT{k}")
                  for k in range(KCP)]

            # PSUM->SBUF copies alternate vector/scalar (gpsimd cannot
            # read PSUM); some scalar table thrash is accepted to keep the
            # qkv psum banks draining fast
            copy_flip = [0]

            def psum_copy(dst, src):
                if copy_flip[0] % 2 == 0:
                    nc.vector.tensor_copy(dst, src)
                else:
                    nc.scalar.copy(dst, src)
                copy_flip[0] += 1

            # ---------------- phase 1: qkvT = Wqkv @ x^T (+bias) ------------
            # tch-major with a rotating x chunk: compute all NF features for
            # one 512-token chunk while the next chunk's DMA is in flight.
            # Score strips for key blocks whose q/k inputs are complete are
            # interleaved after each chunk (range-based hazards make them
            # depend only on the written slices), so the vector/scalar
            # sd-mult + exp pipeline runs during phase 1 instead of after.
            def phase1_tch(tch):
                xt = xt_pool.tile([128, KC, TCH], mm_dt, tag="xt")
                nc.sync.dma_start(
                    xt[:], xT_d[:, tch * XCH:(tch + 1) * XCH])
                if tch == 0:
                    for f in range(1, NF):
                        nc.sync.dma_start(wbig[:, f, :, :],
                                          wqkvT_d[:, f * WCH:(f + 1) * WCH])
                for f in range(NF):
                    ps = psumS.tile([128, TCH], F32, tag="S")
                    for kc in range(KC):
                        nc.tensor.matmul(
                            ps[:],
                            w_sb[(f, kc)],
                            xt[:, kc, :],
                            start=(kc == 0), stop=(kc == KC - 1))
                    dst = qkvT[f][:, tch * TCH:(tch + 1) * TCH]
                    if zero_bias:
                        psum_copy(dst, ps[:])
                    else:
                        nc.scalar.activation(
                            dst, ps[:], AF.Identity, bias=qkvb[:, f:f + 1])

            # ---------------- phase 2: attention per head pair --------------
            shared_decay = [None]

            def make_decay(lh):
                """Decay band for head lh — consts only, can run early."""
                if share_decay and shared_decay[0] is not None:
                    return shared_decay[0]
                dist = work.tile([128, BW], F32, tag="dist",
                                 name=f"dist{lh}")
                nc.vector.tensor_scalar(
                    dist[:], jband[:], rowcol[:], None,
                    AluOpType.subtract)
                nc.scalar.activation(dist[:], dist[:], AF.Abs)
                decay = work.tile([128, BW], F32, tag="decay",
                                  name=f"decay{lh}")
                nc.scalar.activation(decay[:], dist[:], AF.Exp,
                                     scale=neglam[:, lh:lh + 1])
                if share_decay:
                    shared_decay[0] = decay
                return decay

            def head_ctx(lh, decay):
                """Per-head far-field init rows (sfT65): needs v from qkvT."""
                pr, par = lh // 2, lh % 2
                pb = par * 64
                vv = qkvT[2 * P2 + pr]

                # far-field row sums: sfar[ch, k] = EXPM * (tot - band sum)
                vcs = sw.tile([64, NB], F32, tag="vcs", name=f"vcs{lh}")
                nc.vector.tensor_reduce(
                    vcs[:], vv[pb:pb + 64, :].rearrange(
                        "p (k t) -> p k t", k=NB),
                    AX.X, AluOpType.add)
                pad = sw.tile([64, NB + 2 * KSIDE], F32, tag="pad",
                              name=f"pad{lh}")
                nc.gpsimd.memset(pad[:], 0.0)
                nc.gpsimd.tensor_copy(pad[:, KSIDE:KSIDE + NB], vcs[:])
                b5 = sw.tile([64, NB], F32, tag="b5", name=f"b5{lh}")
                nc.gpsimd.tensor_tensor(
                    b5[:], pad[:, 0:NB], pad[:, 1:1 + NB], AluOpType.add)
                for d in range(2, 2 * KSIDE + 1):
                    nc.gpsimd.tensor_tensor(
                        b5[:], b5[:], pad[:, d:d + NB], AluOpType.add)
                tot = stats.tile([64, 1], F32, tag="tot", name=f"tot{lh}")
                nc.vector.tensor_reduce(tot[:], vcs[:], AX.X, AluOpType.add)
                sfar = sw.tile([64, NB], F32, tag="sfar", name=f"sfar{lh}")
                nc.gpsimd.tensor_scalar(
                    sfar[:], b5[:], tot[:], -EXPM,
                    AluOpType.subtract, AluOpType.mult)
                sfT_ps = psumSm.tile([16, 64], F32, tag="small",
                                     name=f"sfT_ps{lh}")
                nc.tensor.transpose(
                    sfT_ps[:NB, :], sfar[:], ident[0:64, 0:64])
                sfT65 = sw.tile([NB, 65], p_dt, tag="sfT65",
                                name=f"sfT65{lh}")
                nc.vector.tensor_copy(sfT65[:, 0:64], sfT_ps[:NB, :])
                nc.gpsimd.tensor_copy(sfT65[:, 64:65], nfexp[:])
                return dict(lh=lh, pb=pb, sfT65=sfT65)

            def make_vnat(pr):
                """v^T for the head pair, 65-col groups [v64 | ones] per
                (block, head): [128, NB*130]."""
                vv = qkvT[2 * P2 + pr]
                vn = work.tile([128, NB * 130], p_dt, tag="vnat",
                               name=f"vnat{pr}")
                nc.gpsimd.memset(vn[:].rearrange(
                    "p (k e) -> p k e", e=65)[:, :, 64:65], 1.0)
                for k in range(NB):
                    tp = psumSm.tile([128, 128], mm_dt, tag="small",
                                     name=f"vtp{pr}_{k}")
                    nc.tensor.transpose(
                        tp[:], vv[:, k * 128:(k + 1) * 128], ident_m)
                    psum_copy(
                        vn[:, k * 130:k * 130 + 130].rearrange(
                            "p (b e) -> p b e", b=2)[:, :, 0:64],
                        tp[:].rearrange("p (b e) -> p b e", b=2))
                return vn

            def strip_pair(decs, pr, c):
                """exp((q.k^T)*decay - MHAT) for key block c, both heads.

                Layout [128 keys, 2*BW]: head h2 at columns h2*BW.."""
                til, tir = band_of(c)
                w = (tir - til + 1) * 128
                off = (til - (c - KSIDE)) * 128
                qq = qkvT[pr]
                kk = qkvT[P2 + pr]
                sdp = work.tile([128, 2 * BW], F32, tag="sdp",
                                name=f"sdp_{pr}_{c}")
                stp = strips_pool.tile([128, 2 * BW], p_dt, tag="strip",
                                       name=f"strip_{pr}_{c}")
                for h2 in range(2):
                    pb = h2 * 64
                    st_ps = psumS.tile([128, TCH], F32, tag="S",
                                       name=f"st_{pr}_{c}_{h2}")
                    nc.tensor.matmul(
                        st_ps[:, :w],
                        kk[pb:pb + 64, c * 128:(c + 1) * 128],
                        qq[pb:pb + 64, til * 128:til * 128 + w],
                        start=True, stop=True)
                    nc.vector.tensor_tensor(
                        sdp[:, h2 * BW:h2 * BW + w], st_ps[:, :w],
                        decs[h2][:, off:off + w], AluOpType.mult)
                if w == BW:
                    nc.scalar.activation(stp[:], sdp[:], AF.Exp,
                                         bias=mhat_b[:])
                else:
                    for h2 in range(2):
                        nc.scalar.activation(
                            stp[:, h2 * BW:h2 * BW + w],
                            sdp[:, h2 * BW:h2 * BW + w], AF.Exp,
                            bias=mhat_b[:])
                return dict(tile=stp, til=til)

            def av_group(hc, h2, vn, strips, g):
                """out^T[65, GQ] for query group g: init with far rows via
                indicator matmul, accumulate band blocks, normalise into
                OT[channels, T]."""
                lh = hc["lh"]
                avps = psumAV.tile([65, GQ], F32, tag="AV",
                                   name=f"av_{lh}_{g}")
                nc.tensor.matmul(
                    avps[:], hc["sfT65"][:],
                    indt[:, g * GQ:(g + 1) * GQ],
                    start=True, stop=False)
                i0 = g * NBG
                cs = [c for c in range(max(0, i0 - KSIDE),
                                       min(NB - 1, i0 + NBG - 1 + KSIDE) + 1)]
                for c in cs:
                    til, tir = band_of(c)
                    ilo, ihi = max(til, i0), min(tir, i0 + NBG - 1)
                    if ilo > ihi:
                        continue
                    sp = strips[c]
                    s0 = h2 * BW + (ilo - sp["til"]) * 128
                    s1 = h2 * BW + (ihi + 1 - sp["til"]) * 128
                    nc.tensor.matmul(
                        avps[:, (ilo - i0) * 128:(ihi + 1 - i0) * 128],
                        vn[:, c * 130 + h2 * 65:c * 130 + h2 * 65 + 65],
                        sp["tile"][:, s0:s1],
                        start=False, stop=(c == cs[-1]))
                # plain DVE reciprocal is ~6ns/element serial — far too slow
                # for a [1, GQ] row; the approx variant (~18 bits) is plenty
                # against the 2e-2 gate and z is a well-scaled positive.
                zrow = stats.tile([1, GQ], F32, tag="zrow",
                                  name=f"z_{lh}_{g}")
                nc.scalar.copy(zrow[:], avps[64:65, :])
                rzrow = stats.tile([1, GQ], F32, tag="rzrow",
                                   name=f"rz_{lh}_{g}")
                nc.vector.reciprocal_approx_fast(rzrow[:], zrow[:])
                rzb = stage_pool.tile([64, GQ], F32, tag="rzb",
                                      name=f"rzb_{lh}_{g}")
                nc.gpsimd.partition_broadcast(rzb[:], rzrow[:], channels=64)
                kc, hh = divmod(lh, 2)
                nc.vector.tensor_tensor(
                    OT[kc][hh * 64:(hh + 1) * 64, g * GQ:(g + 1) * GQ],
                    avps[0:64, :], rzb[:], AluOpType.mult)

            # ---------------- phase 3: yT = projW^T @ OT --------------------
            def proj_group(g):
                for o in range(NO):
                    ps = psumS.tile([128, GQ], F32, tag="S")
                    for kc in range(KCP):
                        nc.tensor.matmul(
                            ps[:],
                            pw_sb[(o, kc)],
                            OT[kc][:, g * GQ:(g + 1) * GQ],
                            start=(kc == 0), stop=(kc == KCP - 1))
                    st = stage_pool.tile([128, GQ], p_dt, tag="stage")
                    if o % 2 == 0:
                        nc.scalar.copy(st[:], ps[:])
                    else:
                        nc.vector.tensor_copy(st[:], ps[:])
                    eng = nc.sync if o % 2 == 0 else nc.gpsimd
                    eng.dma_start(
                        yT_d[o * 128:(o + 1) * 128, g * GQ:(g + 1) * GQ],
                        st[:])

            # ---------------- emission ----------------
            # qkv chunks and score strips interleave (strips for block c run
            # as soon as its q/k token chunks exist), so sd-mult + exp flow
            # through vector/scalar during phase 1.  AV groups and the
            # projection then interleave per query group so output DMAs
            # start early and the tail is short.
            decays = [make_decay(lh) for lh in range(Hpc)]
            all_strips = [dict() for _ in range(P2)]
            cdone = 0
            pair_data = []
            for tch in range(NTCH):
                phase1_tch(tch)
                if tch == NTCH - 1:
                    # vnat + far-field rows first: the gpsimd sfar chain and
                    # the vnat transposes run while vector/scalar drain the
                    # final strips, so AV groups are not gated on them
                    for pr in range(P2):
                        hcs = [head_ctx(2 * pr, decays[2 * pr]),
                               head_ctx(2 * pr + 1, decays[2 * pr + 1])]
                        vn = make_vnat(pr)
                        pair_data.append((hcs, vn, all_strips[pr]))
                cmax = NB if tch == NTCH - 1 else 4 * (tch + 1) - 1
                for c in range(cdone, cmax):
                    for pr in range(P2):
                        all_strips[pr][c] = strip_pair(
                            decays[2 * pr:2 * pr + 2], pr, c)
                cdone = cmax
            for g in range(NG):
                for hcs, vn, strips in pair_data:
                    for h2, hc in enumerate(hcs):
                        av_group(hc, h2, vn, strips, g)
                proj_group(g)

    nc.compile()
    return nc


# ---------------------------------------------------------------------------
# host side
# ---------------------------------------------------------------------------

def _softplus(x):
    x = np.asarray(x, np.float64)
    return np.log1p(np.exp(-np.abs(x))) + np.maximum(x, 0.0)


def make_host_data(x, qkv_w, qkv_b, proj_w, proj_b, lambda_raw,
                   ncores=NCORES, mm_fast=True, bf16_p=True):
    """Returns (cfg, in_maps, assemble(results) -> y)."""
    x = np.asarray(x, np.float32)
    qkv_w = np.asarray(qkv_w, np.float32)
    qkv_b = np.asarray(qkv_b, np.float32)
    proj_w = np.asarray(proj_w, np.float32)
    proj_b = np.asarray(proj_b, np.float32)
    lambda_raw = np.asarray(lambda_raw, np.float32)

    B, T, C = x.shape
    H = lambda_raw.shape[0]
    DH = C // H
    NCH = ncores // B
    Hpc = H // NCH
    P2 = Hpc // 2
    NB = T // 128
    scale = DH ** -0.5

    lam = _softplus(lambda_raw)

    # constant softmax shift: bound on |s| (sampled, with generous margin)
    rng = np.random.default_rng(0)
    idx = rng.choice(B * T, size=min(256, B * T), replace=False)
    xs = x.reshape(B * T, C)[idx]
    qs = (xs @ qkv_w[:C].T).reshape(-1, H, DH)
    ks = (xs @ qkv_w[C:2 * C].T).reshape(-1, H, DH)
    smax = 0.0
    for h in range(H):
        smax = max(smax, float(np.abs(
            (qs[:, h] * scale) @ ks[:, h].T).max()))
    MHAT = float(max(16.0, math.ceil(2.5 * smax + 8.0)))

    lam_min = float(lam.min())
    # band cutoff: beyond the band, |s*d| <= MHAT*exp(-lam*dist) <= 1e-4,
    # so exp(s*d - MHAT) deviates from the far-field exp(-MHAT) by <= 1e-4
    # relative — far below the 2e-2 gate.
    thresh = math.log(max(MHAT, 16.0) / 1e-4)
    KSIDE = _ceil_div(max(1, int(math.ceil(thresh / lam_min)) - 1), 128)
    KSIDE = max(1, KSIDE)
    assert KSIDE == 1, "band wider than 1 tile not supported by this kernel"
    EXPM = math.exp(-MHAT)

    zero_bias = not (qkv_b.any())
    share_decay = bool(np.all(lam == lam[0]))

    cfg = dict(T=T, C=C, DH=DH, Hpc=Hpc, KSIDE=KSIDE, MHAT=MHAT,
               mm_fast=mm_fast, bf16_p=bf16_p, zero_bias=zero_bias,
               share_decay=share_decay)

    NF = 3 * P2
    BW = min(2 * KSIDE + 1, NB) * 128
    pcol = np.arange(128, dtype=np.float32)
    jb = np.broadcast_to(
        np.arange(BW, dtype=np.float32) - KSIDE * 128, (128, BW))
    rc = pcol[:, None]

    # AV group-init indicator [NB, T] and per-block far counts
    ind = np.zeros((NB, T), np.float32)
    for k in range(NB):
        ind[k, k * 128:(k + 1) * 128] = 1.0
    nfar = np.empty((NB, 1), np.float32)
    for i in range(NB):
        kl, kr = max(0, i - KSIDE), min(NB - 1, i + KSIDE)
        nfar[i, 0] = (T - (kr - kl + 1) * 128) * EXPM

    if mm_fast or bf16_p:
        import ml_dtypes
    mm_np = ml_dtypes.bfloat16 if mm_fast else np.float32
    p_np = ml_dtypes.bfloat16 if bf16_p else np.float32

    KC = C // 128
    TCH = 512
    NTCH = T // TCH
    KCP = (Hpc * DH) // 128

    in_maps = []
    for c in range(ncores):
        b, g = divmod(c, NCH)
        hbase = g * Hpc
        # x^T prearranged to SBUF layout [128, (tch kc tcol)] so the device
        # DMA is contiguous per partition
        xT = (x[b].T.reshape(KC, 128, NTCH, TCH).transpose(1, 2, 0, 3)
              .reshape(128, -1).astype(mm_np))
        xT = np.ascontiguousarray(xT)
        wblocks, bblocks = [], []
        for f in range(NF):
            ftype, pr = divmod(f, P2)
            r0 = ftype * C + (hbase + 2 * pr) * DH
            wf = qkv_w[r0:r0 + 128]          # [128, C]
            bf = qkv_b[r0:r0 + 128]
            if ftype == 0:                    # fold score scale into q
                wf = wf * scale
                bf = bf * scale
            wblocks.append(wf.T)
            bblocks.append(bf)
        wqkvT = np.concatenate(wblocks, 1)   # [C, NF*128]
        # -> [128, (f kc m)]
        wqkvT = (wqkvT.reshape(KC, 128, NF, 128).transpose(1, 2, 0, 3)
                 .reshape(128, -1).astype(mm_np))
        wqkvT = np.ascontiguousarray(wqkvT)
        qkvb2d = np.stack(bblocks, 1).astype(np.float32)
        projwT = proj_w[:, hbase * DH:hbase * DH + Hpc * DH].T  # [CPC, C]
        projwT = (projwT.reshape(KCP, 128, C).transpose(1, 0, 2)
                  .reshape(128, -1).astype(mm_np))
        projwT = np.ascontiguousarray(projwT)
        nl = np.broadcast_to(
            (-lam[hbase:hbase + Hpc]).astype(np.float32), (128, Hpc))
        in_maps.append({
            "xT": xT,
            "wqkvT": wqkvT,
            "qkvb2d": np.ascontiguousarray(qkvb2d),
            "projwT": projwT,
            "neglam": np.ascontiguousarray(nl),
            "jband": np.ascontiguousarray(jb),
            "rowcol": np.ascontiguousarray(rc),
            "ident": np.eye(128, dtype=np.float32),
            "ind": ind.astype(p_np),
            "nfexp": nfar.astype(p_np),
        })

    def assemble(results):
        y = np.zeros((B, T, C), np.float32)
        for c in range(ncores):
            b = c // NCH
            y[b] += np.asarray(results[c]["yT"], np.float32).T
        y += proj_b[None, None, :]
        return y

    return cfg, in_maps, assemble


_PROGRAM_CACHE = {}


def kernel(x, qkv_w, qkv_b, proj_w, proj_b, lambda_raw,
           mm_fast=True, bf16_p=True, trace=False):
    cfg, in_maps, assemble = make_host_data(
        x, qkv_w, qkv_b, proj_w, proj_b, lambda_raw,
        mm_fast=mm_fast, bf16_p=bf16_p)
    key = tuple(sorted(cfg.items()))
    if key not in _PROGRAM_CACHE:
        _PROGRAM_CACHE[key] = build_program(cfg)
    nc = _PROGRAM_CACHE[key]
    res = run_bass_kernel_spmd(nc, in_maps, core_ids=list(range(NCORES)),
                               trace=trace)
    out = assemble(res.results)
    if trace:
        kernel.last_results = res
    return out
